# revision 8
# baseline (speedup 1.0000x reference)
"""Trainium2 Bass kernel for PoissonGaussianReadout.

Computation (per reference):
  out[b, n] = elu( sum_c bilinear_sample(x[b, c], mu[n]) * W[n, c] + bias[n] ) + 1

Sharding: data-parallel over batch B=32 across 8 cores (4 images per core).
Every core processes all N=8192 neurons for its 4 images.

Device strategy per core:
  - x is pre-transposed (host) to pixel-major x_t[4096, 4*256] bf16 so that one
    pixel's (b, c) values form a contiguous 2KB row.
  - For each tile of 128 neurons: two indirect DMA gathers (one per y-row of the
    bilinear footprint) fetch row-pairs (x0, x0+1) -> G[128, 4, 4, 256] bf16
    laid out [y, x, b, c].
  - V[n, k, c] = w_k[n] * W[n, c] (4 tensor_scalar ops), then per-b a fused
    scalar_tensor_tensor multiply with accum_out reduces over (k, c) -> z[n, b].
  - Epilogue: z += bias; out = exp(min(z,0)) + max(z,0)  (== elu(z) + 1).
"""

import numpy as np
import ml_dtypes

B, C, H, Wd, N = 32, 256, 64, 64, 8192
NCORES = 8
BL = B // NCORES          # 4 images per core
P = 128                   # partitions / neurons per tile
NT = N // P               # 64 neuron tiles
ROW = BL * C              # 1024 elements per pixel row
NPIX = H * Wd             # 4096

_PROGRAM = None


def _build_program(nt=NT):
    import concourse.bass as bass
    import concourse.mybir as mybir
    import concourse.tile as tile

    bf16 = mybir.dt.bfloat16
    f32 = mybir.dt.float32
    i32 = mybir.dt.int32

    nc = bass.Bass("TRN2")

    xt = nc.dram_tensor("xt", [NPIX, ROW], bf16, kind="ExternalInput")
    # V[n, k, c] = corner_weight_k[n] * W[n, c], host-precomputed, bf16
    wv = nc.dram_tensor("wv", [P, nt * 4 * C], bf16, kind="ExternalInput")
    idx = nc.dram_tensor("idx", [P, nt * 2], i32, kind="ExternalInput")
    bias4 = nc.dram_tensor("bias4", [P, nt * BL], f32, kind="ExternalInput")
    out = nc.dram_tensor("out", [P, nt * BL], f32, kind="ExternalOutput")

    with tile.TileContext(nc) as tc:
        with (
            tc.tile_pool(name="const", bufs=1) as cpool,
            tc.tile_pool(name="gpool", bufs=4) as gpool,
            tc.tile_pool(name="work", bufs=3) as wpool,
        ):
            v_sb = cpool.tile([P, nt * 4 * C], bf16)
            nc.sync.dma_start(v_sb[:], wv[:])
            idx_sb = cpool.tile([P, nt * 2], i32)
            nc.sync.dma_start(idx_sb[:], idx[:])
            bias_sb = cpool.tile([P, nt * BL], f32)
            nc.sync.dma_start(bias_sb[:], bias4[:])
            z_sb = cpool.tile([P, nt * BL], f32)

            # DVE-side join: absorb the const-load DMA waits once, so no
            # TensorScalarPtr instruction ever carries >1 sync wait (HW limit).
            join = cpool.tile([P, 2], f32)
            nc.vector.tensor_copy(join[:, 0:1], v_sb[:, 0:1])
            nc.vector.tensor_copy(join[:, 1:2], bias_sb[:, 0:1])

            half = 2 * BL * C  # one y-row pair: [x(2), b, c]
            for t in range(nt):
                g = gpool.tile([P, 2 * 2 * BL * C], bf16, tag="g")
                # two indirect DMAs (HW supports one index per partition,
                # transfer size = dest row size): rows (p00, p00+1) then
                # (p01, p01+1) -> G layout [y(2), x(2), b, c]
                for j in range(2):
                    nc.gpsimd.indirect_dma_start(
                        out=g[:, j * half : (j + 1) * half],
                        out_offset=None,
                        in_=xt[:, :],
                        in_offset=bass.IndirectOffsetOnAxis(
                            ap=idx_sb[:, 2 * t + j : 2 * t + j + 1], axis=0
                        ),
                    )
                scr = wpool.tile([P, 4, C], bf16, tag="scr")
                g_r = g[:].rearrange("p (y x b c) -> p y x b c", y=2, x=2, b=BL, c=C)
                v_r = v_sb[:, t * 4 * C : (t + 1) * 4 * C].rearrange(
                    "p (k c) -> p k c", k=4, c=C
                )
                for bb in range(BL):
                    nc.vector.scalar_tensor_tensor(
                        out=scr[:].rearrange("p k c -> p k c"),
                        in0=g_r[:, :, :, bb, :].rearrange("p y x c -> p (y x) c"),
                        scalar=1.0,
                        in1=v_r,
                        op0=mybir.AluOpType.mult,
                        op1=mybir.AluOpType.mult,
                        accum_out=z_sb[:, t * BL + bb : t * BL + bb + 1],
                    )

            # epilogue: z += bias;  out = exp(min(z,0)) + max(z,0)
            ze = cpool.tile([P, nt * BL], f32)
            nc.vector.tensor_add(z_sb[:], z_sb[:], bias_sb[:])
            nc.vector.tensor_scalar_min(ze[:], z_sb[:], 0.0)
            nc.scalar.activation(ze[:], ze[:], mybir.ActivationFunctionType.Exp)
            nc.vector.tensor_scalar_max(z_sb[:], z_sb[:], 0.0)
            nc.vector.tensor_add(z_sb[:], z_sb[:], ze[:])
            nc.sync.dma_start(out[:], z_sb[:])

    _split_multi_waits(nc)
    nc.finalize()
    return nc


def _split_multi_waits(nc):
    """The walrus build in this environment only supports ONE sync-wait slot
    per instruction.  Hoist extra waits onto NoOps inserted just before the
    offending instruction (same engine, so sequencer order enforces them)."""
    import concourse.mybir as mybir
    import bass_rust

    for fn in nc.m.functions:
        for blk in fn.blocks:
            new_insts = []
            for ins in blk.instructions:
                si = getattr(ins, "sync_info", None)
                waits = list(si.on_wait) if si is not None else []
                if len(waits) > 1:
                    for j, w in enumerate(waits[:-1]):
                        nop = mybir.InstNoOp(name=f"{ins.name}-w{j}")
                        nop.engine = ins.engine
                        nop.sync_info = bass_rust.SyncInfo(
                            on_wait=[w], on_update=[]
                        )
                        new_insts.append(nop)
                    ins.sync_info = bass_rust.SyncInfo(
                        on_wait=[waits[-1]], on_update=list(si.on_update)
                    )
                new_insts.append(ins)
            blk.instructions[:] = new_insts


def _host_prep(x, mu, W, b):
    bf16 = ml_dtypes.bfloat16

    # --- per-neuron bilinear indices / weights (shared by all cores) ---
    gx = np.clip(mu[:, 0].astype(np.float64), -1.0, 1.0)
    gy = np.clip(mu[:, 1].astype(np.float64), -1.0, 1.0)
    ix = (gx + 1.0) * (Wd * 0.5) - 0.5
    iy = (gy + 1.0) * (H * 0.5) - 0.5
    x0 = np.floor(ix)
    y0 = np.floor(iy)
    wx1 = (ix - x0).astype(np.float32)
    wy1 = (iy - y0).astype(np.float32)
    wx0 = 1.0 - wx1
    wy0 = 1.0 - wy1
    x0i = np.clip(x0.astype(np.int32), 0, Wd - 2)
    y0i = np.clip(y0.astype(np.int32), 0, H - 2)
    p00 = y0i * Wd + x0i            # row index of (y0, x0); pair covers x0, x0+1
    p01 = p00 + Wd                  # row index of (y1, x0)

    def to_pt(a):  # [N, ...] -> [P, NT, ...] with n = t*128 + p
        return np.ascontiguousarray(
            a.reshape(NT, P, *a.shape[1:]).swapaxes(0, 1)
        )

    idx_np = to_pt(np.stack([p00, p01], axis=-1)).reshape(P, NT * 2)
    w4_full = np.stack(
        [wx0 * wy0, wx1 * wy0, wx0 * wy1, wx1 * wy1], axis=-1
    ).astype(np.float32)  # [N, 4]
    v_full = (w4_full[:, :, None] * W[:, None, :]).astype(bf16)  # [N, 4, C]
    wv_np = to_pt(v_full).reshape(P, NT * 4 * C)
    bias_r = to_pt(b.astype(np.float32))  # [P, NT]
    bias4_np = np.ascontiguousarray(
        np.broadcast_to(bias_r[:, :, None], (P, NT, BL))
    ).reshape(P, NT * BL)

    # --- per-core x transpose to pixel-major bf16 ---
    xb = x.astype(bf16).reshape(B, C, NPIX)
    xts = []
    for c in range(NCORES):
        xc = xb[c * BL : (c + 1) * BL]                       # [BL, C, NPIX]
        xt_np = np.ascontiguousarray(xc.transpose(2, 0, 1)).reshape(NPIX, ROW)
        xts.append(xt_np)

    shared = {"wv": wv_np, "idx": idx_np, "bias4": bias4_np}
    return [{"xt": xts[c], **shared} for c in range(NCORES)]


def _run(in_maps, trace=False, **kwargs):
    global _PROGRAM
    from concourse import bass_utils

    if _PROGRAM is None:
        _PROGRAM = _build_program()
    rr = bass_utils.run_bass_kernel_spmd(
        _PROGRAM, in_maps, core_ids=list(range(NCORES)), trace=trace, **kwargs
    )
    outs = []
    for c in range(NCORES):
        o = np.asarray(rr.results[c]["out"], dtype=np.float32)  # [P, NT*BL]
        o = o.reshape(P, NT, BL).transpose(2, 1, 0).reshape(BL, N)
        outs.append(o)
    return np.concatenate(outs, axis=0), rr


def kernel(x, mu, W, b):
    in_maps = _host_prep(x, mu, W, b)
    out, _ = _run(in_maps)
    return out
